# revision 2
# baseline (speedup 1.0000x reference)
"""Trainium2 Bass kernel for nn_Depthwise: binarized depthwise 3x3 conv forward.

    out = dwconv(sign(x), w) + dwconv(x, sign(w)),  stride 1, pad 1
    x: [32, 128, 112, 112] f32, w: [128, 1, 3, 3] f32, alphas: scalars
    (forward value of the STE sign is sign(); alphas only shape gradients).

Strategy (8 NeuronCores, channel-sharded; 16 channels x 32 images per core):
  - TensorE runs as 16 independent 32x32 tiles (tile_position): H=112 is cut
    into 4 strips of 28 output rows (30 input rows with 1-row halos), strip s
    in SBUF partition quadrant s; image-group g (4 images, 453 packed cols
    with zero separators) in PSUM column quadrant g%4.  A banded [30,32]
    lhsT per tile contracts the 3 H-taps; the W-shift of each kernel column
    is a +-1-column PSUM offset.  6 passes (2 convs x 3 kernel cols)
    accumulate in PSUM bank strip+4*(g//4); 16 concurrent tiles give ~4x the
    PE throughput of a full-array banded matmul.
  - bf16 input (host downcast -- sign() is exact in bf16 and the conv ran in
    bf16 anyway) and bf16 output (cast during PSUM evacuation, host upcast)
    halve the HBM traffic; the kernel is DMA-bound.
  - sign(x) as one DVE tensor_scalar (is_gt 0, sub 0.5 -> +-0.5) at 4x bf16
    rate; the factor 2 is folded into the sign-conv bands.  Pad rows/halos
    are zeroed band entries; separator columns are re-zeroed by a strided
    memset.  PSUM evacuation is split between ScalarE and VectorE and
    overlaps the other half's matmuls.
"""

import numpy as np
import ml_dtypes

import concourse.bacc as bacc
import concourse.mybir as mybir
from concourse.tile import TileContext
from concourse.bass_utils import run_bass_kernel_spmd

F32 = mybir.dt.float32
BF16 = mybir.dt.bfloat16

N_CORES = 8
C_TOTAL = 128
NCH = C_TOTAL // N_CORES        # 16 channels per core
N_IMG = 32
H = 112
W = 112
IPG = 4                         # images per PSUM group (453 <= 512 bank)
NG = 8
WP = IPG * (W + 1) + 1          # 453
WB = NG * WP                    # 3624
NS = 4                          # H strips
SM = H // NS                    # 28 output rows per strip
PASS_B = (1, 0, 2)              # kernel-column order per pass (dz = 0,-1,+1)


def build_nc():
    nc = bacc.Bacc(trn_type="TRN2")
    xq = nc.dram_tensor("xq", [NCH, 128, WB], BF16, kind="ExternalInput")
    bands = nc.dram_tensor("bands", [128, NCH * 6 * 32], BF16,
                           kind="ExternalInput")
    out = nc.dram_tensor("out", [NCH, NS, SM, WB], BF16, kind="ExternalOutput")

    with TileContext(nc) as tc:
        with (
            tc.tile_pool(name="bandp", bufs=1) as bandp,
            tc.tile_pool(name="xin", bufs=3) as xpool,
            tc.tile_pool(name="sgn", bufs=3) as spool,
            tc.tile_pool(name="ot", bufs=3) as opool,
            tc.tile_pool(name="ps", bufs=1, space="PSUM") as psp,
        ):
            bt = bandp.tile([128, NCH * 6 * 32], BF16, name="bands", tag="bands")
            nc.sync.dma_start(bt[:, :], bands[:, :])

            for c in range(NCH):
                xt = xpool.tile([128, WB], BF16)
                nc.sync.dma_start(xt[:, :], xq[c])
                st = spool.tile([128, WB], BF16)
                nc.vector.tensor_scalar(
                    st[:, :], xt[:, :], 0.0, 0.5,
                    op0=mybir.AluOpType.is_gt,
                    op1=mybir.AluOpType.subtract)
                nc.vector.memset(
                    st.rearrange("p (g q) -> p g q", q=WP)[:, :, 0:WP:(W + 1)],
                    0.0)
                ot = opool.tile([128, WB], BF16)
                # full-bank pitch (512 f32) keeps partition slices bank-aligned
                pst = [psp.tile([128, 512], F32, name=f"ps{j}", tag=f"ps{j}")
                       for j in range(2 * NS)]
                for h in range(2):
                    for p in range(6):
                        v, bi = p // 3, p % 3
                        rhs_t = xt if v == 0 else st
                        first, last = p == 0, p == 5
                        for s in range(NS):
                            wm = bt[32 * s:32 * s + SM + 2,
                                    (c * 6 + p) * 32:(c * 6 + p) * 32 + 32]
                            ps = pst[s + NS * h]
                            for gg in range(4):
                                j0 = (4 * h + gg) * WP
                                o = ps[32 * gg:32 * gg + 32, 0:WP]
                                r = rhs_t[32 * s:32 * s + SM + 2, :]
                                tp = (32 * s, 32 * gg)
                                if bi == 0:    # b=1 (dz=0)
                                    nc.tensor.matmul(
                                        o[:, 0:WP], wm, r[:, j0:j0 + WP],
                                        start=first, stop=False,
                                        tile_position=tp,
                                        skip_group_check=True)
                                elif bi == 1:  # b=0 (dz=-1)
                                    nc.tensor.matmul(
                                        o[:, 1:WP], wm, r[:, j0:j0 + WP - 1],
                                        start=False, stop=False,
                                        tile_position=tp,
                                        skip_group_check=True)
                                else:          # b=2 (dz=+1)
                                    nc.tensor.matmul(
                                        o[:, 0:WP - 1], wm,
                                        r[:, j0 + 1:j0 + WP],
                                        start=False, stop=last,
                                        tile_position=tp,
                                        skip_group_check=True)
                    for s in range(NS):
                        j = s + NS * h
                        dst = ot[:, j * WP:(j + 1) * WP]
                        if (s + h) % 2 == 0:
                            nc.scalar.copy(dst, pst[j][:, 0:WP])
                        else:
                            nc.vector.tensor_copy(dst, pst[j][:, 0:WP])
                for s in range(NS):
                    nc.gpsimd.dma_start(out[c, s], ot[32 * s:32 * s + SM, :])

    nc.finalize()
    return nc


def make_bands(weight):
    """weight: [NCH, 3, 3] f32 -> [128, NCH*6*32] bf16.

    Block (c, pass p, quadrant q): [32, 32]; entry [m+a, m] = coef[a] for
    out-row m in 0..27, tap a in 0..2.  Passes 0-2: x-conv, coef = sgn(w);
    passes 3-5: sign-conv, coef = 2w (sign tile holds +-0.5).  The zero
    padded halo DATA rows make x-conv edges exact; the sign tile has -0.5
    at pad rows, so sign-conv bands zero those entries (q=0: [0,0];
    q=3: [29,27])."""
    sgn = np.sign(weight)
    B = np.zeros((128, NCH, 6, 32), np.float32)
    for c in range(NCH):
        for p in range(6):
            v, bi = p // 3, p % 3
            b = PASS_B[bi]
            blk = np.zeros((32, 32), np.float32)
            for a in range(3):
                coef = sgn[c, a, b] if v == 0 else 2.0 * weight[c, a, b]
                for m in range(SM):
                    blk[m + a, m] = coef
            for q in range(4):
                blk_q = blk.copy()
                if v == 1:
                    if q == 0:
                        blk_q[0, 0] = 0.0
                    if q == 3:
                        blk_q[SM + 1, SM - 1] = 0.0
                B[32 * q:32 * q + 32, c, p, :] = blk_q
    return np.ascontiguousarray(
        B.reshape(128, NCH * 6 * 32).astype(ml_dtypes.bfloat16))


def pack_x(xc):
    """xc: [NCH, 32, H, W] f32 -> [NCH, 128, WB] bf16 (4 strips x 32 rows)."""
    xg = xc.reshape(NCH, NG, IPG, H, W)
    tmp = np.zeros((NCH, NG, IPG, H, W + 1), np.float32)
    tmp[..., 1:] = xg
    t = tmp.transpose(0, 3, 1, 2, 4).reshape(NCH, H, NG, IPG * (W + 1))
    full = np.zeros((NCH, H + 2, NG, WP), np.float32)
    full[:, 1:H + 1, :, :IPG * (W + 1)] = t
    full = full.reshape(NCH, H + 2, WB)
    strips = np.zeros((NCH, NS, 32, WB), np.float32)
    for s in range(NS):
        strips[:, s, :SM + 2] = full[:, SM * s:SM * s + SM + 2]
    return np.ascontiguousarray(
        strips.reshape(NCH, 128, WB).astype(ml_dtypes.bfloat16))


def unpack_out(o):
    """o: [NCH, 4, 28, WB] bf16 -> [NCH, 32, H, W] f32.

    DRAM row (q, r) of col block j = s + 4h holds output H-row 28s+r of
    image group g = 4h+q; block col layout [z i0 z i1 z i2 z i3 z]."""
    t = np.asarray(o, dtype=np.float32).reshape(NCH, 4, SM, NG, WP)
    t = t[..., :IPG * (W + 1)].reshape(NCH, 4, SM, NG, IPG, W + 1)[..., 1:]
    t = t.reshape(NCH, 4, SM, 2, 4, IPG, W)       # [c, q, r, h, s, i, w]
    t = t.transpose(0, 3, 1, 5, 4, 2, 6)          # [c, h, q, i, s, r, w]
    return t.reshape(NCH, N_IMG, H, W)


def kernel(x, weight, alpha_x=None, alpha_w=None):
    """Full inputs in, full output out. Shards channels across 8 cores."""
    x = np.ascontiguousarray(np.asarray(x, dtype=np.float32))
    weight = np.asarray(weight, dtype=np.float32).reshape(C_TOTAL, 3, 3)

    X = x.transpose(1, 0, 2, 3)  # [C, N, H, W]
    in_maps = []
    for k in range(N_CORES):
        cs = slice(NCH * k, NCH * (k + 1))
        in_maps.append({
            "xq": pack_x(X[cs]),
            "bands": make_bands(weight[cs]),
        })

    nc = build_nc()
    res = run_bass_kernel_spmd(nc, in_maps, core_ids=list(range(N_CORES)))

    got = np.empty((N_IMG, C_TOTAL, H, W), np.float32)
    for k in range(N_CORES):
        o = unpack_out(res.results[k]["out"])  # [NCH, N_IMG, H, W]
        got[:, NCH * k:NCH * (k + 1)] = o.transpose(1, 0, 2, 3)
    return got
